# revision 1
# baseline (speedup 1.0000x reference)
"""Multi-head attention (B=4, N=1568, C=768, H=12) on 8 TRN2 NeuronCores.

Sharding: query-parallel. Core c handles batch b = c // 2 and query half
half = c % 2 (784 query tokens). Each core computes K/V projections for the
full 1568 tokens of its batch (duplicated across the pair), Q projection
for its 784 tokens, full attention for all 12 heads over its queries, and
the output projection. No cross-core communication.

Host-side tricks:
  - tokens are rotated per core so its own query half sits at columns 0:784
    of xT; the key order is then a (core-dependent) permutation, which
    softmax attention is invariant to. This removes the separate xqT input.
  - v_bias is folded into the projection bias:
      out = (attn + 1 (x) v_bias) @ proj_w + proj_b
          = attn @ proj_w + (proj_b + v_bias @ proj_w)
  - the softmax 1/sqrt(D) scale is folded into the exp activation's scale.

Device pipeline (per core), heads processed in PAIRS (2ft, 2ft+1):
  scores: the pair's two QK^T matmuls go to disjoint PE row-groups
    (kT rows 0:64 vs 64:128) so they run concurrently; outputs land in two
    joint PSUM tiles (sJ0: both heads' q0 chunk, sJ1: both heads' q1 chunk)
  exp: one ACT instruction per joint tile -> ex [mt, 1568] bf16
  PV ("form B"): ex q-tiles (<=128 queries) are the matmul STATIONARY and
    [V | ones] (65 cols) streams -> po[q, 65] accumulates over key tiles;
    col 64 is the softmax denominator, per-PARTITION (per query!), so
    normalize is a cheap per-partition reciprocal + tensor_scalar multiply
  transpose: normalized [q, 64] head outputs go back to feature-major via
    PE identity-matmul transpose, evicted into attn (bf16) for the proj
  V is projected per (pair, 4-key-tile batch) just in time; K/Q projections
    for pair p+1 interleave into pair p's steps as PE gap fillers; a short
    dummy-matmul stream at kernel start warms the PE HAM clock gate.
"""

import numpy as np
import ml_dtypes

B, N, C = 4, 1568, 768
H = 12
D = 64
NQ = N // 2          # 784 queries per core
SCALE = D ** -0.5
N_CORES = 8
KT = [128] * 12 + [32]          # key tiles (sum = 1568)
TCH = [(0, 392), (392, 392), (784, 392), (1176, 392)]  # token chunks (K/Q proj)
# query tiles for form-B PV: (ex column offset, width, output q offset).
# ex is laid out head-contiguous (A: cols 0:784, B: 784:1568); 7 uniform
# 112-wide tiles per head (112 != 128 keeps FWL off the ex-stationary path)
QT_A = [(112 * i, 112, 112 * i) for i in range(7)]
QT_B = [(784 + 112 * i, 112, 112 * i) for i in range(7)]

_cache = {}


def _build_program():
    import concourse.mybir as mybir
    from concourse import bacc
    from concourse.tile import TileContext

    f32 = mybir.dt.float32
    f32r = mybir.dt.float32r
    bf16 = mybir.dt.bfloat16
    Exp = mybir.ActivationFunctionType.Exp

    nc = bacc.Bacc("TRN2", target_bir_lowering=False, debug=False,
                   num_devices=N_CORES)

    xT_d = nc.dram_tensor("xT", [C, N], bf16, kind="ExternalInput")
    wqk_d = nc.dram_tensor("wqk", [C, 2 * C], bf16, kind="ExternalInput")
    wv_d = nc.dram_tensor("wv", [C, C], bf16, kind="ExternalInput")
    wp_d = nc.dram_tensor("wproj", [C, C], bf16, kind="ExternalInput")
    qb_d = nc.dram_tensor("qb", [128, 6], f32, kind="ExternalInput")
    pb_d = nc.dram_tensor("pb", [128, 6], f32, kind="ExternalInput")
    id_d = nc.dram_tensor("ident", [128, 128], bf16, kind="ExternalInput")
    out_d = nc.dram_tensor("outT", [C, NQ], f32, kind="ExternalOutput")

    with TileContext(nc) as tc:
        persist_cm = tc.tile_pool(name="persist", bufs=1)
        persist = persist_cm.__enter__()
        kT = [persist.tile([128, N], bf16, tag=f"kT{j}", name=f"kT{j}")
              for j in range(6)]
        qT = [persist.tile([128, NQ], bf16, tag=f"qT{j}", name=f"qT{j}")
              for j in range(6)]
        # V for all 13 key tiles: [tt][pair][head-of-pair][65] along free dim
        # (64 V cols + ones col per head)
        vbig = persist.tile([128, 13 * 780], bf16, tag="vbig", name="vbig")
        attn = [persist.tile([128, NQ], bf16, tag=f"at{j}", name=f"at{j}")
                for j in range(6)]
        qb_sb = persist.tile([128, 6], f32, tag="qb")
        pb_sb = persist.tile([128, 6], f32, tag="pb")
        id_sb = persist.tile([128, 128], bf16, tag="ident")
        nc.sync.dma_start(out=qb_sb, in_=qb_d[:])
        nc.sync.dma_start(out=pb_sb, in_=pb_d[:])
        nc.sync.dma_start(out=id_sb, in_=id_d[:])
        # ones columns: every 65th col of vbig starting at 64
        nc.vector.memset(
            vbig.rearrange("p (t e) -> p t e", e=65)[:, :, 64:65], 1.0)

        wpp_cm = tc.tile_pool(name="wpp", bufs=1)
        wpp = wpp_cm.__enter__()
        wp_sb = [wpp.tile([128, C], bf16, tag=f"wp{j}", name=f"wp{j}")
                 for j in range(6)]

        phA_cm = tc.tile_pool(name="phA", bufs=1)
        phA = phA_cm.__enter__()
        xT = [phA.tile([128, N], bf16, tag=f"xT{j}", name=f"xTs{j}")
              for j in range(6)]
        wqk = [phA.tile([128, 2 * C], bf16, tag=f"wqk{j}", name=f"wqks{j}")
               for j in range(6)]
        wv = [phA.tile([128, C], bf16, tag=f"wv{j}", name=f"wvs{j}")
              for j in range(6)]
        for j in range(6):
            nc.sync.dma_start(out=xT[j], in_=xT_d[j * 128:(j + 1) * 128, :])
            nc.sync.dma_start(out=wqk[j][:, C:2 * C],
                              in_=wqk_d[j * 128:(j + 1) * 128, C:2 * C])
        for j in range(6):
            nc.sync.dma_start(out=wqk[j][:, 0:C],
                              in_=wqk_d[j * 128:(j + 1) * 128, 0:C])
            nc.sync.dma_start(out=wv[j], in_=wv_d[j * 128:(j + 1) * 128, :])

        # PSUM pools: sJ0 2 + sJ1 2 + po 2 + psA 2 = 8 banks
        psS_cm = tc.tile_pool(name="psS", bufs=1, space="PSUM")
        psS = psS_cm.__enter__()
        psO_cm = tc.tile_pool(name="psO", bufs=1, space="PSUM")
        psO = psO_cm.__enter__()
        psA_cm = tc.tile_pool(name="psA", bufs=2, space="PSUM")
        psA = psA_cm.__enter__()
        phB_cm = tc.tile_pool(name="phB", bufs=5)
        phB = phB_cm.__enter__()
        phBn_cm = tc.tile_pool(name="phBn", bufs=3)
        phBn = phBn_cm.__enter__()

        def emit_k(ft, chunks):
            # chunk-pair inner loop: consecutive matmuls share the wqk
            # stationary so the second skips its weight load
            pss = [psA.tile([128, 512], f32, tag="psA", name=f"k{ft}_{ci}")
                   for ci in chunks]
            for j in range(6):
                for ps, ci in zip(pss, chunks):
                    (t0, tw) = TCH[ci]
                    nc.tensor.matmul(
                        ps[:, 0:tw],
                        wqk[j][:, C + ft * 128:C + ft * 128 + 128],
                        xT[j][:, t0:t0 + tw],
                        start=(j == 0), stop=(j == 5),
                        skip_group_check=True,
                    )
            for ps, ci in zip(pss, chunks):
                (t0, tw) = TCH[ci]
                nc.vector.tensor_copy(kT[ft][:, t0:t0 + tw], ps[:, 0:tw])

        def emit_q(ft, chunks):
            pss = [psA.tile([128, 512], f32, tag="psA", name=f"q{ft}_{ci}")
                   for ci in chunks]
            for j in range(6):
                for ps, ci in zip(pss, chunks):
                    (t0, tw) = TCH[ci]
                    nc.tensor.matmul(
                        ps[:, 0:tw],
                        wqk[j][:, ft * 128:ft * 128 + 128],
                        xT[j][:, t0:t0 + tw],
                        start=(j == 0), stop=(j == 5),
                        skip_group_check=True,
                    )
            for ps, ci in zip(pss, chunks):
                (t0, tw) = TCH[ci]
                nc.vector.tensor_scalar(
                    out=qT[ft][:, t0:t0 + tw], in0=ps[:, 0:tw],
                    scalar1=qb_sb[:, ft:ft + 1], scalar2=None,
                    op0=mybir.AluOpType.add,
                )

        def emit_v(p, tt0):
            # V for pair p, key tiles tt0..tt0+3 (4-tile batch), into one
            # psA tile then one strided eviction into vbig
            tts = [t for t in range(tt0, min(tt0 + 4, 13))]
            ps = psA.tile([128, 512], f32, tag="psA", name=f"v{p}_{tt0}")
            for i, tt in enumerate(tts):
                mt = KT[tt]
                for j in range(6):
                    # single bank-wide start (see emit_pv)
                    nc.tensor.matmul(
                        ps[0:mt, i * 128:i * 128 + 128],
                        xT[j][:, tt * 128:tt * 128 + mt],
                        wv[j][:, p * 128:(p + 1) * 128],
                        start=(j == 0 and i == 0),
                        stop=(j == 5 and i == len(tts) - 1),
                        skip_group_check=True,
                    )
            # evict: psum [128, i*128 + h*64 + e] -> vbig col
            # tt*780 + p*130 + h*65 + e
            src = ps.rearrange("q (i h e) -> q i h e", i=4, h=2)[
                0:128, 0:len(tts), :, :]
            dst = vbig.rearrange("q (t pp h e) -> q t pp h e", pp=6, h=2, e=65)[
                0:128, tt0:tt0 + len(tts), p, :, 0:64]
            nc.vector.tensor_copy(dst, src)

        def emit_scores(ft, tt):
            mt = KT[tt]
            sJ0 = psS.tile([128, 1024], f32, tag="sJ0", name=f"s0_{ft}_{tt}")
            sJ1 = psS.tile([128, 1024], f32, tag="sJ1", name=f"s1_{ft}_{tt}")
            ks = slice(tt * 128, tt * 128 + mt)
            # A's two q-chunks back-to-back (second reuses loaded weights),
            # then B's two (concurrent on row-group 64:128)
            nc.tensor.matmul(sJ0[0:mt, 0:512], kT[ft][0:64, ks],
                             qT[ft][0:64, 0:512], start=True, stop=True)
            nc.tensor.matmul(sJ1[0:mt, 0:272], kT[ft][0:64, ks],
                             qT[ft][0:64, 512:784], start=True, stop=True)
            nc.tensor.matmul(sJ0[0:mt, 512:1024], kT[ft][64:128, ks],
                             qT[ft][64:128, 0:512], start=True, stop=True)
            nc.tensor.matmul(sJ1[0:mt, 512:784], kT[ft][64:128, ks],
                             qT[ft][64:128, 512:784], start=True, stop=True)
            # ex layout: head A = cols 0:784, head B = cols 784:1568
            ex = phB.tile([128, 1568], bf16, tag="ex", name=f"ex{ft}_{tt}")
            ex2 = ex.rearrange("p (b q) -> p b q", b=2)
            nc.scalar.activation(out=ex2[0:mt, :, 0:512],
                                 in_=sJ0.rearrange("p (b q) -> p b q", b=2)[0:mt],
                                 func=Exp, scale=SCALE)
            nc.scalar.activation(
                out=ex2[0:mt, :, 512:784],
                in_=sJ1.rearrange("p (b q) -> p b q", b=2)[0:mt, :, 0:272],
                func=Exp, scale=SCALE)
            return ex

        def emit_pv(p, tt, ex, po_pair):
            mt = KT[tt]
            v5 = vbig.rearrange("q (t pp h e) -> q t pp h e", pp=6, h=2, e=65)
            for hh in range(2):
                po = po_pair[hh]
                qts = QT_A if hh == 0 else QT_B
                vh = v5[0:mt, tt, p, hh, :]
                for i, (c0, qw, _) in enumerate(qts):
                    # start marks the WHOLE 2KB psum bank pending-zero, so
                    # only the very first matmul into this po bank may set
                    # it; later qtile groups overwrite-on-first-touch via
                    # the per-element has_written bits.
                    nc.tensor.matmul(
                        po[0:qw, i * 65:(i + 1) * 65],
                        ex[0:mt, c0:c0 + qw],
                        vh,
                        start=(tt == 0 and i == 0),
                        stop=(tt == 12 and i == 6),
                        skip_group_check=True,
                    )

        def evict_po(p, po_pair):
            # fast po->SBUF copy frees the (single-buffered) po psum bank so
            # the next pair's PV isn't blocked behind the normalize chain
            pz_pair = []
            for hh in range(2):
                pz = phBn.tile([128, 455], f32, tag=f"pz{hh}",
                               name=f"pz{p}_{hh}")
                nc.vector.tensor_copy(pz, po_pair[hh][:, 0:455])
                pz_pair.append(pz)
            return pz_pair

        def emit_normalize(p, pz_pair):
            # pz[q, 65i:65i+64] = PV, pz[q, 65i+64] = denominator (per q!)
            for hh in range(2):
                ft, fo = p, hh * 64
                pz = pz_pair[hh]
                qts = QT_A if hh == 0 else QT_B
                rec = phBn.tile([128, 7], f32, tag=f"rec{hh}", name=f"rc{p}_{hh}")
                nc.vector.reciprocal_approx_fast(
                    out=rec,
                    in_=pz.rearrange("q (i e) -> q i e", e=65)[:, 0:7, 64])
                aq = phBn.tile([128, 448], bf16, tag=f"aq{hh}",
                               name=f"aq{p}_{hh}")
                for i, (c0, qw, _) in enumerate(qts):
                    nc.vector.tensor_scalar(
                        out=aq[0:qw, i * 64:i * 64 + 64],
                        in0=pz[0:qw, i * 65:i * 65 + 64],
                        scalar1=rec[0:qw, i:i + 1], scalar2=None,
                        op0=mybir.AluOpType.mult,
                    )
                # transpose [q, 64] tiles back to feature-major via PE;
                # batch qtiles 0-3 then 4-6 into psA-bank-sized groups
                for g0, gn, q0 in ((0, 4, 0), (4, 3, 448)):
                    pt = psA.tile([128, 512], bf16, tag="psA",
                                  name=f"tr{p}_{hh}_{g0}")
                    for i in range(g0, g0 + gn):
                        qw = qts[i][1]
                        qo = qts[i][2] - q0
                        nc.tensor.transpose(
                            pt[0:64, qo:qo + qw],
                            aq[0:qw, i * 64:i * 64 + 64],
                            id_sb[0:qw, 0:qw])
                    gw = qts[g0 + gn - 1][2] + qts[g0 + gn - 1][1] - q0
                    nc.vector.tensor_copy(
                        attn[ft][fo:fo + 64, q0:q0 + gw], pt[0:64, 0:gw])

        with nc.named_scope("qkv"):
            # dummy matmuls on the first-arrived DMA tile warm the PE's HAM
            # clock gate (~3.4us of activity) while the rest of the weights
            # stream in; outputs are discarded
            for w in range(8):
                wps = psA.tile([128, 512], f32, tag="psA", name=f"warm{w}")
                nc.tensor.matmul(wps[:, 0:512], xT[0][:, 0:128],
                                 xT[0][:, 0:512], start=True, stop=True)
            emit_k(0, [0, 1])
            emit_q(0, [0, 1])
            emit_k(0, [2, 3])

        # K/Q projection fill schedule: during pair p emit pieces of
        # K/Q(p+1); keyed by step tt
        fill = {p: {} for p in range(6)}
        for p in range(5):
            fill[p][1] = lambda ft=p + 1: emit_k(ft, [0, 1])
            fill[p][6] = lambda ft=p + 1: emit_q(ft, [0])
            fill[p][8] = lambda ft=p + 1: emit_k(ft, [2, 3])
            fill[p][12] = lambda ft=p + 1: emit_q(ft, [1])

        with nc.named_scope("attn"):
            prev = None  # (p, pz_pair) pending normalize
            for p in range(6):
                po_pair = [psO.tile([128, 512], f32, tag=f"po{hh}",
                                    name=f"po{p}_{hh}") for hh in range(2)]
                for tt in range(13):
                    ex = emit_scores(p, tt)
                    if tt % 4 == 0:
                        emit_v(p, tt)
                    emit_pv(p, tt, ex, po_pair)
                    if tt in fill[p]:
                        fill[p][tt]()
                    if tt == 2 and prev is not None:
                        emit_normalize(*prev)
                        prev = None
                    if p == 4 and tt == 11:
                        for j in range(6):
                            nc.sync.dma_start(
                                out=wp_sb[j],
                                in_=wp_d[j * 128:(j + 1) * 128, :])
                prev = (p, evict_po(p, po_pair))
            emit_normalize(*prev)

        phBn_cm.__exit__(None, None, None)
        phB_cm.__exit__(None, None, None)
        psA_cm.__exit__(None, None, None)
        psO_cm.__exit__(None, None, None)
        psS_cm.__exit__(None, None, None)
        phA_cm.__exit__(None, None, None)

        # ================= output projection =================
        with (
            nc.named_scope("proj"),
            tc.tile_pool(name="psP", bufs=4, space="PSUM") as psP,
            tc.tile_pool(name="phC", bufs=3) as phC,
        ):
            for ot in range(6):
                for (q0, qw) in ((0, 512), (512, 272)):
                    ps = psP.tile([128, 512], f32, tag="psP")
                    for j in range(6):
                        nc.tensor.matmul(
                            ps[:, 0:qw],
                            wp_sb[j][:, ot * 128:(ot + 1) * 128],
                            attn[j][:, q0:q0 + qw],
                            start=(j == 0), stop=(j == 5),
                        )
                    ob = phC.tile([128, 512], f32, tag="ob")
                    nc.vector.tensor_scalar(
                        out=ob[:, 0:qw], in0=ps[:, 0:qw],
                        scalar1=pb_sb[:, ot:ot + 1], scalar2=None,
                        op0=mybir.AluOpType.add,
                    )
                    nc.sync.dma_start(
                        out=out_d[ot * 128:(ot + 1) * 128, q0:q0 + qw],
                        in_=ob[:, 0:qw])

        wpp_cm.__exit__(None, None, None)
        persist_cm.__exit__(None, None, None)

    nc.compile()
    return nc


def _get_program():
    if "nc" not in _cache:
        _cache["nc"] = _build_program()
    return _cache["nc"]


def _make_in_maps(x, qkv_w, q_bias, v_bias, proj_w, proj_b):
    wqk = np.ascontiguousarray(qkv_w[:, :2 * C])      # [C, 2C] (q cols, k cols)
    wv = np.ascontiguousarray(qkv_w[:, 2 * C:])       # [C, C]
    qb = np.zeros((128, 6), np.float32)
    qb[:, :] = q_bias.reshape(6, 128).T
    pb_eff = proj_b + v_bias @ proj_w                  # fold v_bias into proj
    pb = np.zeros((128, 6), np.float32)
    pb[:, :] = pb_eff.reshape(6, 128).T
    ident = np.eye(128, dtype=ml_dtypes.bfloat16)

    in_maps = []
    for c in range(N_CORES):
        b, half = c // 2, c % 2
        # rotate tokens so this core's query half sits at columns 0:NQ;
        # key order becomes a permutation, which softmax attention is
        # invariant to
        xT = np.ascontiguousarray(
            np.roll(x[b].T, -half * NQ, axis=1)).astype(ml_dtypes.bfloat16)
        in_maps.append({
            "xT": xT, "wqk": wqk.astype(ml_dtypes.bfloat16),
            "wv": wv.astype(ml_dtypes.bfloat16),
            "wproj": proj_w.astype(ml_dtypes.bfloat16), "qb": qb, "pb": pb,
            "ident": ident,
        })
    return in_maps


def kernel(x, qkv_w, q_bias, v_bias, proj_w, proj_b):
    from concourse.bass_utils import run_bass_kernel_spmd

    x = np.asarray(x, dtype=np.float32)
    qkv_w = np.asarray(qkv_w, dtype=np.float32)
    q_bias = np.asarray(q_bias, dtype=np.float32)
    v_bias = np.asarray(v_bias, dtype=np.float32)
    proj_w = np.asarray(proj_w, dtype=np.float32)
    proj_b = np.asarray(proj_b, dtype=np.float32)

    nc = _get_program()
    in_maps = _make_in_maps(x, qkv_w, q_bias, v_bias, proj_w, proj_b)
    _cache["in_maps"] = in_maps

    res = run_bass_kernel_spmd(nc, in_maps, list(range(N_CORES)))
    out = np.empty((B, N, C), np.float32)
    for c in range(N_CORES):
        b, half = c // 2, c % 2
        out[b, half * NQ:(half + 1) * NQ, :] = res.results[c]["outT"].T
    return out

